# revision 1
# baseline (speedup 1.0000x reference)
"""Graph-transformer layer (masked dense attention + FFN) on 8 trn2 cores.

Sharding (per spec hint): core c handles batch b = c//2 and query rows
[(c%2)*2048, (c%2)*2048+2048) of that batch.  K/V and all weights are
replicated within the 2-core batch group.

Per-core pipeline (fp32 end to end):
  phase A: x blocks -> x^T via PE transpose; K^T [h,n], V [n,h], Q^T [h,q]
           projections.  Biases are folded in exactly as rank-1 accumulate
           matmuls (ones-row x bias-row) into the same PSUM group.
  phase B: per 128-row query tile:
             scores chunk = Q^T.T @ K^T chunk (PSUM, 512 cols)
             P = exp(scores/16)           (ACT, PSUM->SBUF)
             P *= adj; rowsum partials    (DVE tensor_tensor_reduce)
             P^T blocks via PE transpose  -> AV accumulate (PSUM)
             O = AV * (1/rowsum)          (ACT scale-by-AP)
             O^T via PE transpose -> FF1^T = relu(W1^T O^T + b1) -> Y -> DMA
  The softmax skips max-subtraction: scores/16 stays O(5) for any sane
  input so fp32 exp cannot overflow, and softmax is shift-invariant.
  Masked entries are exactly zeroed by the adj multiply, so row sums and
  AV match the reference's -1e9 masking.
"""

import os
from contextlib import ExitStack

import numpy as np

B, N, D, H = 4, 4096, 256, 256
NQ = N // 2  # query rows per core
P = 128  # SBUF partitions
NCHUNK = 512  # scores free-dim chunk = one fp32 PSUM bank
NCORES = 8

_CACHE = {}


def _build():
    import concourse.bass as bass
    import concourse.bacc as bacc
    import concourse.mybir as mybir
    from concourse.tile import TileContext

    f32 = mybir.dt.float32
    i32 = mybir.dt.int32
    AF = mybir.ActivationFunctionType

    n_qt = NQ // P  # 16 query tiles
    n_nb = N // P  # 32 key blocks
    n_ck = N // NCHUNK  # 8 score chunks per row tile
    DT = D // P  # 2 contraction tiles over D
    HT = H // P  # 2 tiles over H

    nc = bacc.Bacc("TRN2", target_bir_lowering=False)

    x_d = nc.dram_tensor("xb", [N, D], f32, kind="ExternalInput").ap()
    xq_d = nc.dram_tensor("xq", [NQ, D], f32, kind="ExternalInput").ap()
    adj_d = nc.dram_tensor("adjs", [NQ, N], i32, kind="ExternalInput").ap()
    w_d = {
        nm: nc.dram_tensor(nm, [256, 256], f32, kind="ExternalInput").ap()
        for nm in ("Wq", "Wk", "Wv", "W1", "W2")
    }
    b_d = {
        nm: nc.dram_tensor(nm, [1, 256], f32, kind="ExternalInput").ap()
        for nm in ("bq", "bk", "bv", "b1", "b2")
    }
    ident_d = nc.dram_tensor("ident_in", [P, P], f32, kind="ExternalInput").ap()
    ones_d = nc.dram_tensor("ones_in", [1, NCHUNK], f32, kind="ExternalInput").ap()
    out_d = nc.dram_tensor("out", [NQ, D], f32, kind="ExternalOutput").ap()

    with ExitStack() as ctx:
        tc = ctx.enter_context(TileContext(nc))
        const = ctx.enter_context(tc.tile_pool(name="const", bufs=1))
        kT_p = ctx.enter_context(tc.tile_pool(name="kT", bufs=1))
        v_p = ctx.enter_context(tc.tile_pool(name="v", bufs=1))
        qT_p = ctx.enter_context(tc.tile_pool(name="qT", bufs=1))
        adj_p = ctx.enter_context(tc.tile_pool(name="adj", bufs=2))
        prow_p = ctx.enter_context(tc.tile_pool(name="prow", bufs=1))
        negm_p = ctx.enter_context(tc.tile_pool(name="negm", bufs=1))
        xin_p = ctx.enter_context(tc.tile_pool(name="xin", bufs=3))
        xtb_p = ctx.enter_context(tc.tile_pool(name="xtb", bufs=3))
        pt_p = ctx.enter_context(tc.tile_pool(name="pt", bufs=4))
        ot_p = ctx.enter_context(tc.tile_pool(name="ot", bufs=3))
        ff_p = ctx.enter_context(tc.tile_pool(name="ff", bufs=3))
        y_p = ctx.enter_context(tc.tile_pool(name="y", bufs=2))
        st_p = ctx.enter_context(tc.tile_pool(name="st", bufs=2))
        tp_ps = ctx.enter_context(tc.tile_pool(name="tp_ps", bufs=3, space="PSUM"))
        mm_ps = ctx.enter_context(tc.tile_pool(name="mm_ps", bufs=4, space="PSUM"))

        # ---- constants ----
        ident = const.tile([P, P], f32)
        nc.sync.dma_start(ident[:], ident_d[:])
        ones = const.tile([1, NCHUNK], f32)
        nc.sync.dma_start(ones[:], ones_d[:])
        w_sb = {}
        for nm in ("Wq", "Wk", "Wv", "W1", "W2"):
            w = const.tile([P, DT, 256], f32, tag=f"w_{nm}")
            for i in range(DT):
                nc.sync.dma_start(w[:, i, :], w_d[nm][i * P : (i + 1) * P, :])
            w_sb[nm] = w
        b_sb = {}
        for nm in ("bq", "bk", "bv", "b1", "b2"):
            bt = const.tile([1, 256], f32, tag=f"b_{nm}")
            nc.sync.dma_start(bt[:], b_d[nm][:])
            b_sb[nm] = bt

        # ---- persistent activations ----
        kT = kT_p.tile([P, HT, N], f32)  # K^T: [h%128, h//128, n]
        v_sb = v_p.tile([P, n_nb, H], f32)  # V: [n%128, n//128, h]
        qT = qT_p.tile([P, HT, NQ], f32)  # Q^T: [h%128, h//128, q]

        def xT_block(src, blk):
            """DMA a 128-row x block and PE-transpose to [d, dt, 128]."""
            xin = xin_p.tile([P, D], f32)
            nc.sync.dma_start(xin[:], src[blk * P : (blk + 1) * P, :])
            xtb = xtb_p.tile([P, DT, P], f32)
            for dt in range(DT):
                ps = tp_ps.tile([P, P], f32, tag="tps")
                nc.tensor.matmul(ps[:], xin[:, dt * P : (dt + 1) * P], ident[:],
                                 start=True, stop=True)
                nc.vector.tensor_copy(xtb[:, dt, :], ps[:])
            return xtb

        def proj_T(dst, dst_sl, wname, bname, xtb):
            """dst[:, ht, dst_sl] = (W^T x + b)^T columns for one 128 block."""
            for ht in range(HT):
                ps = tp_ps.tile([P, P], f32, tag="tps")
                for dt in range(DT):
                    nc.tensor.matmul(
                        ps[:],
                        w_sb[wname][:, dt, ht * P : (ht + 1) * P],
                        xtb[:, dt, :],
                        start=(dt == 0),
                        stop=False,
                    )
                nc.tensor.matmul(
                    ps[:],
                    b_sb[bname][0:1, ht * P : (ht + 1) * P],
                    ones[0:1, 0:P],
                    start=False,
                    stop=True,
                )
                nc.scalar.copy(dst[:, ht, dst_sl], ps[:])

        # ---- phase A ----
        for blk in range(n_nb):
            xtb = xT_block(x_d, blk)
            sl = slice(blk * P, (blk + 1) * P)
            proj_T(kT, sl, "Wk", "bk", xtb)
            psv = mm_ps.tile([P, NCHUNK], f32, tag="mm")
            for dt in range(DT):
                nc.tensor.matmul(
                    psv[:, 0:H],
                    xtb[:, dt, :],
                    w_sb["Wv"][:, dt, :],
                    start=(dt == 0),
                    stop=False,
                )
            nc.tensor.matmul(
                psv[:, 0:H], ones[0:1, 0:P], b_sb["bv"][0:1, :], start=False, stop=True
            )
            nc.scalar.copy(v_sb[:, blk, :], psv[:, 0:H])
        for blk in range(n_qt):
            xtb = xT_block(xq_d, blk)
            proj_T(qT, slice(blk * P, (blk + 1) * P), "Wq", "bq", xtb)

        # ---- phase B ----
        inv_sqrt_h = 1.0 / np.sqrt(np.float32(H))
        for qt in range(n_qt):
            qsl = slice(qt * P, (qt + 1) * P)
            adj_t = adj_p.tile([P, N], i32)
            nc.sync.dma_start(adj_t[:], adj_d[qsl, :])
            prow = prow_p.tile([P, N], f32)
            negm = negm_p.tile([P, N], f32)
            # negmask = (adj - 1) * 1e9 : 0 where edge, -1e9 where masked
            nc.gpsimd.tensor_scalar(
                out=negm[:], in0=adj_t[:], scalar1=-1, scalar2=1e9,
                op0=mybir.AluOpType.add, op1=mybir.AluOpType.mult,
            )
            sums = st_p.tile([P, n_ck], f32, tag="sums")
            for ci in range(n_ck):
                csl = slice(ci * NCHUNK, (ci + 1) * NCHUNK)
                ps = mm_ps.tile([P, NCHUNK], f32, tag="mm")
                for ht in range(HT):
                    nc.tensor.matmul(
                        ps[:],
                        qT[:, ht, qsl],
                        kT[:, ht, csl],
                        start=(ht == 0),
                        stop=(ht == HT - 1),
                    )
                nc.vector.tensor_add(prow[:, csl], ps[:], negm[:, csl])
                nc.scalar.activation(
                    prow[:, csl], prow[:, csl], AF.Exp, scale=inv_sqrt_h,
                    accum_out=sums[:, ci : ci + 1],
                )
            l_all = st_p.tile([P, 1], f32, tag="l_all")
            nc.vector.reduce_sum(l_all[:], sums[:], axis=mybir.AxisListType.X)
            rl = st_p.tile([P, 1], f32, tag="rl")
            nc.vector.reciprocal(rl[:], l_all[:])

            o_ps = mm_ps.tile([P, NCHUNK], f32, tag="mm")
            for blk in range(n_nb):
                tps = tp_ps.tile([P, P], f32, tag="tps")
                nc.tensor.matmul(
                    tps[:], prow[:, blk * P : (blk + 1) * P], ident[:],
                    start=True, stop=True,
                )
                ptb = pt_p.tile([P, P], f32)
                if blk % 2 == 0:
                    nc.vector.tensor_copy(ptb[:], tps[:])
                else:
                    nc.scalar.copy(ptb[:], tps[:])
                nc.tensor.matmul(
                    o_ps[:, 0:H],
                    ptb[:],
                    v_sb[:, blk, :],
                    start=(blk == 0),
                    stop=(blk == n_nb - 1),
                )
            o_sb = ot_p.tile([P, H], f32, tag="o_sb")
            nc.scalar.mul(o_sb[:], o_ps[:, 0:H], rl[:])

            # FFN: FF1^T[h2, q] = relu(W1^T O^T + b1), Y = FF1 W2 + b2
            oT = []
            for ht in range(HT):
                tps = tp_ps.tile([P, P], f32, tag="tps")
                nc.tensor.matmul(tps[:], o_sb[:, ht * P : (ht + 1) * P], ident[:],
                                 start=True, stop=True)
                ot = ot_p.tile([P, P], f32, tag="oT_sb")
                nc.vector.tensor_copy(ot[:], tps[:])
                oT.append(ot)
            ff1 = []
            for ht2 in range(HT):
                fps = tp_ps.tile([P, P], f32, tag="tps")
                for ht in range(HT):
                    nc.tensor.matmul(
                        fps[:],
                        w_sb["W1"][:, ht, ht2 * P : (ht2 + 1) * P],
                        oT[ht][:],
                        start=(ht == 0),
                        stop=False,
                    )
                nc.tensor.matmul(
                    fps[:],
                    b_sb["b1"][0:1, ht2 * P : (ht2 + 1) * P],
                    ones[0:1, 0:P],
                    start=False,
                    stop=True,
                )
                ff = ff_p.tile([P, P], f32)
                nc.scalar.activation(ff[:], fps[:], AF.Relu)
                ff1.append(ff)
            y_ps = mm_ps.tile([P, NCHUNK], f32, tag="mm")
            for ht2 in range(HT):
                nc.tensor.matmul(
                    y_ps[:, 0:D],
                    ff1[ht2][:],
                    w_sb["W2"][:, ht2, :],
                    start=(ht2 == 0),
                    stop=False,
                )
            nc.tensor.matmul(
                y_ps[:, 0:D], ones[0:1, 0:P], b_sb["b2"][0:1, :], start=False,
                stop=True,
            )
            y_sb = y_p.tile([P, D], f32)
            nc.scalar.copy(y_sb[:], y_ps[:, 0:D])
            nc.sync.dma_start(out_d[qsl, :], y_sb[:])

    return nc


def _get_nc():
    if "nc" not in _CACHE:
        nc = _build()
        nc.finalize()  # Bacc: splits multi-sem waits to satisfy HW 1-wait limit
        _CACHE["nc"] = nc
    return _CACHE["nc"]


def kernel(x, adj, Wq, bq, Wk, bk, Wv, bv, W1, b1, W2, b2):
    from concourse.bass_utils import run_bass_kernel_spmd

    x = np.ascontiguousarray(np.asarray(x, dtype=np.float32))
    adj = np.ascontiguousarray(np.asarray(adj, dtype=np.int32))
    weights = {
        "Wq": np.ascontiguousarray(np.asarray(Wq, np.float32)),
        "Wk": np.ascontiguousarray(np.asarray(Wk, np.float32)),
        "Wv": np.ascontiguousarray(np.asarray(Wv, np.float32)),
        "W1": np.ascontiguousarray(np.asarray(W1, np.float32)),
        "W2": np.ascontiguousarray(np.asarray(W2, np.float32)),
        "bq": np.ascontiguousarray(np.asarray(bq, np.float32).reshape(1, 256)),
        "bk": np.ascontiguousarray(np.asarray(bk, np.float32).reshape(1, 256)),
        "bv": np.ascontiguousarray(np.asarray(bv, np.float32).reshape(1, 256)),
        "b1": np.ascontiguousarray(np.asarray(b1, np.float32).reshape(1, 256)),
        "b2": np.ascontiguousarray(np.asarray(b2, np.float32).reshape(1, 256)),
    }
    nc = _get_nc()
    in_maps = []
    for c in range(NCORES):
        b, half = c // 2, c % 2
        q0 = half * NQ
        m = {
            "xb": x[b],
            "xq": np.ascontiguousarray(x[b, q0 : q0 + NQ]),
            "adjs": np.ascontiguousarray(adj[b, q0 : q0 + NQ]),
        }
        m.update(weights)
        m["ident_in"] = np.eye(P, dtype=np.float32)
        m["ones_in"] = np.ones((1, NCHUNK), dtype=np.float32)
        in_maps.append(m)
    global _last_in_maps
    _last_in_maps = in_maps
    res = run_bass_kernel_spmd(nc, in_maps, list(range(NCORES)))
    out = np.empty((B, N, D), dtype=np.float32)
    for c in range(NCORES):
        b, half = c // 2, c % 2
        q0 = half * NQ
        out[b, q0 : q0 + NQ] = res.results[c]["out"]
    return out

